# revision 1
# baseline (speedup 1.0000x reference)
"""2-layer IndRNN (diagonal recurrence) + linear head on 8 trn2 NeuronCores.

Strategy (data-parallel over batch, 32 rows/core, 2 chunks of 16):
  - Feature-major activation layout [h_inner=partition, (o, t, b)=free].
  - GEMM-0: pre0 = W0 @ x per 16-timestep block, f32r matmul (fp32 in, FP22
    multiply, fp32 PSUM accumulate), bias fused into the PSUM->SBUF copy.
  - Recurrences keep the fp32 PRE-activation state z_t = u*relu(z_{t-1}) +
    pre_t; the relu is fused into the next step's scalar_tensor_tensor
    ((z max 0) mult u_bcast), so each step is exactly 2 DVE ops.
  - Layer-0 state lives in-place in a fp32 pre0 ring; one block-wise ACT
    relu+convert materializes the bf16 h0 operand for GEMM-1.
  - GEMM-1 is all-bf16 (weights resident in SBUF, 64KB/partition),
    accumulated over 16 k-tiles in PSUM, bias fused into the copy to a
    small bf16 ring consumed by recurrence 1.
  - Head: relu+f32r convert of the last z1 state, then a 16-step
    accumulated [128,1]x[128,16] matmul + bias.
Host side only reorders/shards numpy inputs; all FLOPs run on device.
"""

import numpy as np

B, T, I, H = 256, 100, 128, 2048
NCORES = 8
BL = B // NCORES            # batch rows per core
BC = 16                     # batch rows per chunk
NCH = BL // BC              # chunks per core
NO = H // 128               # 16 h-tiles
TBLKS = [(0, 16), (16, 16), (32, 16), (48, 16), (64, 16), (80, 16), (96, 4)]

_CACHE = {}


def _build():
    import concourse.tile as tile
    from concourse import bacc, mybir

    f32 = mybir.dt.float32
    bf16 = mybir.dt.bfloat16
    f32r = mybir.dt.float32r
    RELU = mybir.ActivationFunctionType.Relu
    IDENT = mybir.ActivationFunctionType.Identity
    MAX = mybir.AluOpType.max
    MULT = mybir.AluOpType.mult

    nc = bacc.Bacc(None, target_bir_lowering=False)

    xT_d = nc.dram_tensor("xT", [128, NCH, T, BC], f32r, kind="ExternalInput")
    w0T_d = nc.dram_tensor("w0T", [128, NO, 128], f32r, kind="ExternalInput")
    w1T_d = nc.dram_tensor("w1T", [128, NO, NO, 128], bf16, kind="ExternalInput")
    u0f_d = nc.dram_tensor("u0f", [128, NO, BC], f32, kind="ExternalInput")
    u1f_d = nc.dram_tensor("u1f", [128, NO, BC], f32, kind="ExternalInput")
    b0_d = nc.dram_tensor("b0t", [128, NO], f32, kind="ExternalInput")
    b1_d = nc.dram_tensor("b1t", [128, NO], f32, kind="ExternalInput")
    lw_d = nc.dram_tensor("lwt", [128, NO], f32r, kind="ExternalInput")
    lb_d = nc.dram_tensor("lbt", [1, 1], f32, kind="ExternalInput")
    out_d = nc.dram_tensor("out", [1, BL], f32, kind="ExternalOutput")

    with tile.TileContext(nc) as tc:
        with (
            tc.tile_pool(name="const", bufs=1) as const,
            tc.tile_pool(name="xb", bufs=3) as xb,
            tc.tile_pool(name="p0", bufs=2) as p0p,
            tc.tile_pool(name="h0", bufs=6) as h0p,
            tc.tile_pool(name="h0tail", bufs=1) as h0tp,
            tc.tile_pool(name="ring", bufs=3) as ring,
            tc.tile_pool(name="tmp", bufs=6) as tmp,
            tc.tile_pool(name="h1s", bufs=2) as h1sp,
            tc.tile_pool(name="ps0", bufs=2, space="PSUM") as ps0,
            tc.tile_pool(name="ps1", bufs=3, space="PSUM") as ps1,
        ):
            w0T = const.tile([128, NO, 128], f32r, tag="w0T")
            w1T = const.tile([128, NO, NO, 128], bf16, tag="w1T")
            u0f = const.tile([128, NO, BC], f32, tag="u0f")
            u1f = const.tile([128, NO, BC], f32, tag="u1f")
            b0t = const.tile([128, NO], f32, tag="b0t")
            b1t = const.tile([128, NO], f32, tag="b1t")
            lwt = const.tile([128, NO], f32r, tag="lwt")
            lbt = const.tile([1, 1], f32, tag="lbt")
            outs = const.tile([1, BL], f32, tag="outs")

            nc.sync.dma_start(out=w0T[:], in_=w0T_d[:])
            nc.sync.dma_start(out=u0f[:], in_=u0f_d[:])
            nc.sync.dma_start(out=u1f[:], in_=u1f_d[:])
            nc.sync.dma_start(out=b0t[:], in_=b0_d[:])
            nc.sync.dma_start(out=b1t[:], in_=b1_d[:])
            nc.sync.dma_start(out=lwt[:], in_=lw_d[:])
            nc.sync.dma_start(out=lbt[:], in_=lb_d[:])

            all_h0 = {}
            all_sts = {}

            all_p0 = {}

            def emit_g0_block(c, nb):
                # ---- GEMM-0 + recurrence 0 + bf16 h0 block nb ------------
                p0blks = all_p0.setdefault(c, [])
                h0blks = all_h0.setdefault(c, [])
                if True:
                    t0, TB = TBLKS[nb]
                    xt = xb.tile([128, 16, BC], f32r, tag="xb")
                    nc.sync.dma_start(out=xt[:, :TB], in_=xT_d[:, c, t0:t0 + TB])
                    pb = p0p.tile([128, NO, TB, BC], f32, tag="p0")
                    p0blks.append(pb)
                    for m in range(NO):
                        ps = ps0.tile([128, 16, BC], f32, tag="ps0")
                        nc.tensor.matmul(
                            ps[:, :TB], w0T[:, m], xt[:, :TB],
                            start=True, stop=True,
                        )
                        nc.scalar.activation(
                            pb[:, m], ps[:, :TB], IDENT,
                            bias=b0t[:, m:m + 1], scale=1.0,
                        )
                    # recurrence 0 over this block, in place (z state)
                    for trel in range(TB):
                        t = t0 + trel
                        if t == 0:
                            continue  # z_0 = pre_0 already in place
                        cur = pb[:, :, trel]
                        pbb, pt = ((t - 1) >> 4), ((t - 1) & 15)
                        prev = p0blks[pbb][:, :, pt]
                        tm = tmp.tile([128, NO, BC], f32, tag="tmp")
                        nc.vector.scalar_tensor_tensor(
                            tm[:], prev, 0.0, u0f[:], MAX, MULT,
                        )
                        nc.vector.tensor_add(cur, tm[:], cur)
                    # block-wise relu + bf16 convert -> GEMM-1 operand
                    pool = h0p if TB == 16 else h0tp
                    hb = pool.tile([128, NO, TB, BC], bf16,
                                   tag="h0" if TB == 16 else "h0t")
                    h0blks.append(hb)
                    nc.scalar.activation(hb[:], pb[:], RELU)

            def emit_g1(c, lo=0, hi=None):
                # ---- GEMM-1 + recurrence 1, block by block ---------------
                h0blks = all_h0[c]
                if c not in all_sts:
                    st_a = h1sp.tile([128, NO, BC], f32, tag="h1s")
                    st_b = h1sp.tile([128, NO, BC], f32, tag="h1s")
                    all_sts[c] = (st_a, st_b)
                sts = all_sts[c]
                hi = len(TBLKS) if hi is None else hi
                for nb, (t0, TB) in list(enumerate(TBLKS))[lo:hi]:
                    rb = ring.tile([128, NO, TB, BC], bf16, tag="ring")
                    for mg in range(4):
                        ps = ps1.tile([128, 4, 16, BC], f32, tag="ps1")
                        for ml in range(4):
                            m = mg * 4 + ml
                            for k in range(NO):
                                nc.tensor.matmul(
                                    ps[:, ml, :TB],
                                    w1T[:, k, m],
                                    h0blks[nb][:, k],
                                    start=(k == 0), stop=(k == NO - 1),
                                )
                        for ml in range(4):
                            m = mg * 4 + ml
                            nc.scalar.activation(
                                rb[:, m], ps[:, ml, :TB], IDENT,
                                bias=b1t[:, m:m + 1], scale=1.0,
                            )
                    for trel in range(TB):
                        t = t0 + trel
                        pre = rb[:, :, trel]
                        cur = sts[t & 1][:]
                        if t == 0:
                            nc.vector.tensor_copy(cur, pre)
                        else:
                            prev = sts[(t - 1) & 1][:]
                            tm = tmp.tile([128, NO, BC], f32, tag="tmp")
                            nc.vector.scalar_tensor_tensor(
                                tm[:], prev, 0.0, u1f[:], MAX, MULT,
                            )
                            nc.vector.tensor_add(cur, tm[:], pre)

                if hi < len(TBLKS):
                    return
                # ---- head: out[b] = lin_w . relu(z1_T) + lin_b -----------
                h1h = h1sp.tile([128, NO, BC], f32r, tag="h1h")
                nc.scalar.activation(h1h[:], sts[(T - 1) & 1][:], RELU)
                ph = ps0.tile([128, 16, BC], f32, tag="ps0")
                for o in range(NO):
                    nc.tensor.matmul(
                        ph[0:1, 0], lwt[:, o:o + 1], h1h[:, o],
                        start=(o == 0), stop=(o == NO - 1),
                    )
                nc.scalar.activation(
                    outs[0:1, c * BC:(c + 1) * BC], ph[0:1, 0], IDENT,
                    bias=lbt[0:1, 0:1], scale=1.0,
                )

            for nb in range(len(TBLKS)):
                emit_g0_block(0, nb)
            for kb in range(NO):
                nc.sync.dma_start(out=w1T[:, kb], in_=w1T_d[:, kb])
            emit_g1(0, 0, 2)
            for nb in range(len(TBLKS)):
                emit_g0_block(1, nb)
                if 2 + nb < len(TBLKS):
                    emit_g1(0, 2 + nb, 3 + nb)
            emit_g1(0, 2 + len(TBLKS))
            emit_g1(1)

            nc.sync.dma_start(out=out_d[:], in_=outs[:])

    nc.compile()
    return nc


def _get_nc():
    if "nc" not in _CACHE:
        _CACHE["nc"] = _build()
    return _CACHE["nc"]


def _trunc22(a):
    return (np.ascontiguousarray(a).view(np.int32) & np.int32(~0x3FF)).view(np.float32)


def _prep_shared(W0, b0, u0, W1, b1, u1, lin_w, lin_b):
    import ml_dtypes

    w0T = _trunc22(np.ascontiguousarray(W0.T)).reshape(128, NO, 128)
    w1T = np.ascontiguousarray(
        W1.reshape(NO, 128, NO, 128).transpose(3, 2, 0, 1)
    ).astype(ml_dtypes.bfloat16)
    u0f = np.ascontiguousarray(
        np.broadcast_to(u0.reshape(NO, 128).T[:, :, None], (128, NO, BC))
    ).astype(np.float32)
    u1f = np.ascontiguousarray(
        np.broadcast_to(u1.reshape(NO, 128).T[:, :, None], (128, NO, BC))
    ).astype(np.float32)
    b0t = np.ascontiguousarray(b0.reshape(NO, 128).T)
    b1t = np.ascontiguousarray(b1.reshape(NO, 128).T)
    lwt = _trunc22(np.ascontiguousarray(lin_w.reshape(NO, 128).T))
    lbt = np.ascontiguousarray(lin_b.reshape(1, 1))
    return dict(w0T=w0T, w1T=w1T, u0f=u0f, u1f=u1f,
                b0t=b0t, b1t=b1t, lwt=lwt, lbt=lbt)


def make_in_maps(x, W0, b0, u0, W1, b1, u1, lin_w, lin_b):
    shared = _prep_shared(
        np.asarray(W0, np.float32), np.asarray(b0, np.float32),
        np.asarray(u0, np.float32), np.asarray(W1, np.float32),
        np.asarray(b1, np.float32), np.asarray(u1, np.float32),
        np.asarray(lin_w, np.float32), np.asarray(lin_b, np.float32),
    )
    x = np.asarray(x, np.float32)
    in_maps = []
    for core in range(NCORES):
        xc = x[core * BL:(core + 1) * BL]            # (BL, T, I)
        xT = _trunc22(np.ascontiguousarray(
            xc.reshape(NCH, BC, T, 128).transpose(3, 0, 2, 1)
        ))                                           # (128, NCH, T, BC)
        in_maps.append({"xT": xT, **shared})
    return in_maps


def kernel(x, W0, b0, u0, W1, b1, u1, lin_w, lin_b):
    from concourse.bass_utils import run_bass_kernel_spmd

    nc = _get_nc()
    in_maps = make_in_maps(x, W0, b0, u0, W1, b1, u1, lin_w, lin_b)
    try:
        res = run_bass_kernel_spmd(nc, in_maps, list(range(NCORES)))
    except Exception:
        res = run_bass_kernel_spmd(nc, in_maps, list(range(NCORES)))
    return np.concatenate([r["out"][0] for r in res.results])



# revision 6
# speedup vs baseline: 3.4516x; 3.4516x over previous
"""2-layer IndRNN (diagonal recurrence) + linear head on 8 trn2 NeuronCores.

Data-parallel over batch: 32 rows/core, all 32 in the free dim (no chunk
split; free size 512 per op).

Numerics (validated ~1.45e-2 rel err vs fp64, gate 2e-2):
  - GEMM-0 (x @ W0^T) in fp16, 1 cyc/row. PSUM f32, drained with bias to an
    fp16 pre/z ring on GpSimd (tensor_scalar_add).
  - Recurrences keep fp16 PRE-activation state z_t in place in the ring;
    each step is stt((z_{t-1} max 0) mult u[f32]) + fp16 tensor_add (DVE 2x).
  - h0 = relu(z0) emitted blockwise by ACT as fp8e4m3 scaled by 2^4
    (exact power-2 folding), per m0-pair so GEMM-1 can start early.
  - GEMM-1 (h0 @ W1^T) in fp8e4m3 DoubleRow: weights scaled by 2^13, 8
    k-pair matmuls (256-deep each) per m-tile -> 2x bf16 throughput. The
    PSUM drain applies scale 2^-17 and bias on ACT, writing the fp16 rb
    ring consumed in-place by recurrence 1.
  - Head: relu(z1_99) fp16, 16-step accumulated [128,1]x[128,32] matmul,
    + lin_b on the final ACT copy.
Host side only reorders/converts numpy inputs; all FLOPs run on device.
"""

import numpy as np

B, T, I, H = 256, 100, 128, 2048
NCORES = 8
BL = B // NCORES            # 32 batch rows per core, all in free dim
NO = H // 128               # 16 hidden tiles
KP = NO // 2                # 8 DoubleRow k-pairs
SH = 8.0                    # h0 fp8 scale (power of 2); kept low because the
                            # fp8 matmul path NaNs when |PSUM| nears fp16 max
TBLKS = [(0, 16), (16, 16), (32, 16), (48, 16), (64, 16), (80, 16), (96, 4)]

_CACHE = {}


def _build(sw_scale):
    import concourse.tile as tile
    from concourse import bacc, mybir

    f32 = mybir.dt.float32
    f16 = mybir.dt.float16
    f8 = mybir.dt.float8e4
    RELU = mybir.ActivationFunctionType.Relu
    IDENT = mybir.ActivationFunctionType.Identity
    MAX = mybir.AluOpType.max
    MULT = mybir.AluOpType.mult
    DR = mybir.MatmulPerfMode.DoubleRow
    SC = 1.0 / (sw_scale * SH)  # GEMM-1 drain descale (exact power of 2)

    nc = bacc.Bacc(None, target_bir_lowering=False)

    xT_d = nc.dram_tensor("xT", [128, T, BL], f16, kind="ExternalInput")
    w0T_d = nc.dram_tensor("w0T", [128, NO, 128], f16, kind="ExternalInput")
    w1T_d = nc.dram_tensor("w1T", [128, KP, 2, NO, 128], f8, kind="ExternalInput")
    u0f_d = nc.dram_tensor("u0f", [128, NO, BL], f32, kind="ExternalInput")
    u1f_d = nc.dram_tensor("u1f", [128, NO, BL], f32, kind="ExternalInput")
    b0_d = nc.dram_tensor("b0t", [128, NO], f32, kind="ExternalInput")
    b1_d = nc.dram_tensor("b1t", [128, NO], f32, kind="ExternalInput")
    lw_d = nc.dram_tensor("lwt", [128, NO], f16, kind="ExternalInput")
    lb_d = nc.dram_tensor("lbt", [1, 1], f32, kind="ExternalInput")
    out_d = nc.dram_tensor("out", [1, BL], f32, kind="ExternalOutput")

    NB = len(TBLKS)

    with tile.TileContext(nc) as tc:
        with (
            tc.tile_pool(name="const", bufs=1) as const,
            tc.tile_pool(name="pb", bufs=3) as pbp,
            tc.tile_pool(name="hb", bufs=2) as hbp,
            tc.tile_pool(name="rb", bufs=3) as rbp,
            tc.tile_pool(name="tm", bufs=6) as tmp,
            tc.tile_pool(name="ps0", bufs=3, space="PSUM") as ps0,
            tc.tile_pool(name="ps1", bufs=3, space="PSUM") as ps1,
        ):
            xt = const.tile([128, T, BL], f16, tag="xt")
            w0T = const.tile([128, NO, 128], f16, tag="w0T")
            w1T = const.tile([128, KP, 2, NO, 128], f8, tag="w1T")
            u0f = const.tile([128, NO, BL], f32, tag="u0f")
            u1f = const.tile([128, NO, BL], f32, tag="u1f")
            b0t = const.tile([128, NO], f32, tag="b0t")
            b1t = const.tile([128, NO], f32, tag="b1t")
            lwt = const.tile([128, NO], f16, tag="lwt")
            lbt = const.tile([1, 1], f32, tag="lbt")
            outs = const.tile([1, BL], f32, tag="outs")
            h1h = const.tile([128, NO, BL], f16, tag="h1h")

            nc.sync.dma_start(out=w0T[:], in_=w0T_d[:])
            nc.sync.dma_start(out=xt[:], in_=xT_d[:])
            nc.sync.dma_start(out=u0f[:], in_=u0f_d[:])
            nc.sync.dma_start(out=u1f[:], in_=u1f_d[:])
            nc.sync.dma_start(out=b0t[:], in_=b0_d[:])
            nc.sync.dma_start(out=b1t[:], in_=b1_d[:])
            nc.sync.dma_start(out=lwt[:], in_=lw_d[:])
            nc.sync.dma_start(out=lbt[:], in_=lb_d[:])

            pbs, hbs, rbs = {}, {}, {}

            def g0(b):
                # GEMM-0 block + GpSimd drain (bias) into the fp16 z ring
                t0, TB = TBLKS[b]
                pb = pbp.tile([128, NO, 16, BL], f16, tag="pb")
                pbs[b] = pb
                for m0 in range(NO):
                    ps = ps0.tile([128, 16, BL], f32, tag="ps0")
                    nc.tensor.matmul(
                        ps[:, :TB], w0T[:, m0], xt[:, t0:t0 + TB],
                        start=True, stop=True,
                    )
                    nc.scalar.activation(
                        pb[:, m0, :TB], ps[:, :TB], IDENT,
                        bias=b0t[:, m0:m0 + 1], scale=1.0,
                    )

            def r0(b):
                t0, TB = TBLKS[b]
                pb = pbs[b]
                for trel in range(TB):
                    t = t0 + trel
                    if t == 0:
                        continue
                    prev = (pb[:, :, trel - 1] if trel
                            else pbs[b - 1][:, :, 15])
                    tm = tmp.tile([128, NO, BL], f16, tag="tm")
                    nc.vector.scalar_tensor_tensor(
                        tm[:], prev, 0.0, u0f[:], MAX, MULT,
                    )
                    cur = pb[:, :, trel]
                    nc.vector.tensor_add(cur, tm[:], cur)

            def h0(b):
                t0, TB = TBLKS[b]
                hb = hbp.tile([128, NO, 16, BL], f8, tag="hb")
                hbs[b] = hb
                for kp in range(KP):
                    nc.scalar.activation(
                        hb[:, 2 * kp:2 * kp + 2, :TB],
                        pbs[b][:, 2 * kp:2 * kp + 2, :TB],
                        RELU, scale=SH,
                    )

            def g1(b):
                t0, TB = TBLKS[b]
                rb = rbp.tile([128, NO, 16, BL], f16, tag="rb")
                rbs[b] = rb
                hb = hbs[b]
                for m in range(NO):
                    ps = ps1.tile([128, 16, BL], f32, tag="ps1")
                    for kp in range(KP):
                        nc.tensor.matmul(
                            ps[:, :TB],
                            w1T[:, kp, :, m],
                            hb[:, 2 * kp:2 * kp + 2, :TB],
                            start=(kp == 0), stop=(kp == KP - 1),
                            perf_mode=DR,
                        )
                    nc.scalar.activation(
                        rb[:, m, :TB], ps[:, :TB], IDENT,
                        bias=b1t[:, m:m + 1], scale=SC,
                    )

            def r1(b):
                t0, TB = TBLKS[b]
                rb = rbs[b]
                for trel in range(TB):
                    t = t0 + trel
                    if t == 0:
                        continue
                    prev = (rb[:, :, trel - 1] if trel
                            else rbs[b - 1][:, :, 15])
                    tm = tmp.tile([128, NO, BL], f16, tag="tm")
                    nc.vector.scalar_tensor_tensor(
                        tm[:], prev, 0.0, u1f[:], MAX, MULT,
                    )
                    cur = rb[:, :, trel]
                    nc.vector.tensor_add(cur, tm[:], cur)

            def head():
                lt0, lTB = TBLKS[NB - 1]
                nc.scalar.activation(
                    h1h[:], rbs[NB - 1][:, :, lTB - 1], RELU, scale=1.0,
                )
                ph = ps0.tile([128, 16, BL], f32, tag="ps0")
                for m in range(NO):
                    nc.tensor.matmul(
                        ph[0:1, 0], lwt[:, m:m + 1], h1h[:, m],
                        start=(m == 0), stop=(m == NO - 1),
                    )
                nc.scalar.activation(
                    outs[0:1, :], ph[0:1, 0], IDENT,
                    bias=lbt[0:1, 0:1], scale=1.0,
                )

            # ---- software pipeline ----
            # slot s: PE does g0(s+1) then g1(s); DVE does r1(s-1) then
            # r0(s+1); ACT does h0(s) then drains of g1(s); GpSimd drains
            # g0(s+1).
            g0(0)
            for kp in range(KP):
                nc.sync.dma_start(out=w1T[:, kp], in_=w1T_d[:, kp])
            r0(0)
            g0(1)
            h0(0)
            r0(1)
            for b in range(NB):
                if b + 2 < NB:
                    g0(b + 2)
                g1(b)
                if b >= 1:
                    r1(b - 1)
                if b + 2 < NB:
                    r0(b + 2)
                if b + 1 < NB:
                    h0(b + 1)
            r1(NB - 1)
            head()

            nc.sync.dma_start(out=out_d[:], in_=outs[:])

    nc.compile()
    return nc


def _get_nc(sw_scale=8192.0):
    key = ("nc", sw_scale)
    if key not in _CACHE:
        _CACHE[key] = _build(sw_scale)
    return _CACHE[key]


def _prep_shared(W0, b0, u0, W1, b1, u1, lin_w, lin_b):
    import ml_dtypes

    # power-of-2 weight scale keeping max|W1|*sw < 448 (e4m3 max)
    wmax = float(np.abs(W1).max()) or 1.0
    sw = float(2.0 ** np.floor(np.log2(448.0 / wmax)))
    sw = min(sw, 2048.0)

    w0T = np.ascontiguousarray(W0.T.reshape(128, NO, 128)).astype(np.float16)
    w1T = np.ascontiguousarray(
        W1.reshape(NO, 128, KP, 2, 128).transpose(4, 2, 3, 0, 1) * sw
    ).astype(ml_dtypes.float8_e4m3fn)
    u0f = np.ascontiguousarray(
        np.broadcast_to(u0.reshape(NO, 128).T[:, :, None], (128, NO, BL))
    ).astype(np.float32)
    u1f = np.ascontiguousarray(
        np.broadcast_to(u1.reshape(NO, 128).T[:, :, None], (128, NO, BL))
    ).astype(np.float32)
    b0t = np.ascontiguousarray(b0.reshape(NO, 128).T).astype(np.float32)
    b1t = np.ascontiguousarray(b1.reshape(NO, 128).T).astype(np.float32)
    lwt = np.ascontiguousarray(lin_w.reshape(NO, 128).T).astype(np.float16)
    lbt = np.ascontiguousarray(lin_b.reshape(1, 1)).astype(np.float32)
    return sw, dict(w0T=w0T, w1T=w1T, u0f=u0f, u1f=u1f,
                    b0t=b0t, b1t=b1t, lwt=lwt, lbt=lbt)


def make_in_maps(x, W0, b0, u0, W1, b1, u1, lin_w, lin_b):
    sw, shared = _prep_shared(
        np.asarray(W0, np.float32), np.asarray(b0, np.float32),
        np.asarray(u0, np.float32), np.asarray(W1, np.float32),
        np.asarray(b1, np.float32), np.asarray(u1, np.float32),
        np.asarray(lin_w, np.float32), np.asarray(lin_b, np.float32),
    )
    x = np.asarray(x, np.float32)
    in_maps = []
    for core in range(NCORES):
        xc = x[core * BL:(core + 1) * BL]            # (BL, T, I)
        xT = np.ascontiguousarray(xc.transpose(2, 1, 0)).astype(np.float16)
        in_maps.append({"xT": xT, **shared})
    return sw, in_maps


def kernel(x, W0, b0, u0, W1, b1, u1, lin_w, lin_b):
    from concourse.bass_utils import run_bass_kernel_spmd

    sw, in_maps = make_in_maps(x, W0, b0, u0, W1, b1, u1, lin_w, lin_b)
    nc = _get_nc(sw)
    try:
        res = run_bass_kernel_spmd(nc, in_maps, list(range(NCORES)))
    except Exception:
        res = run_bass_kernel_spmd(nc, in_maps, list(range(NCORES)))
    return np.concatenate([r["out"][0] for r in res.results])
